# revision 29
# baseline (speedup 1.0000x reference)
"""Trainium2 Bass kernel for GNN copy_src -> segment-mean (dst-sharded, 8 cores).

Strategy
--------
- Partition dst nodes (and their incoming edges) across 8 NeuronCores:
  core c owns dst rows [c*6250, (c+1)*6250).
- Host-side "inspector" pass (numpy): bucket each core's edges by
  128-slot dst block, split each block's edges into two runs by src range
  (dma_gather indices are int16, so the 50000-row table is addressed as
  two halves), pad each run to a multiple of 128 with harmless dummy
  edges (src index 0, slot sentinel -1). Per-dst reciprocal degree is
  also computed host-side (it only depends on dst).
- Device kernel (identical SPMD program on all 8 cores):
  * dma_gather batches of source rows (512 B each) from the full
    author_emb table in HBM into SBUF. Gather calls round-robin over
    4 SWDGE queues: each queue's descriptor generation runs on its own
    Q7 core pair (queue q -> cores 2q, 2q+1), so up to 4 gathers
    generate descriptors concurrently instead of serializing on one
    core pair.
  * per 128-edge group, build a one-hot matrix H[edge, slot] on the DVE
    via is_equal(slot_value, iota_row); dummy edges give all-zero rows.
  * TensorE matmuls accumulate H^T @ G (feature sums) in PSUM per
    128-slot block.
  * per block: multiply by the host-provided reciprocal degree column,
    DMA the [128, 128] result tile to the output shard.
- Host gathers the 8 output shards into the full [50000, 128] output.
"""

import os
import sys

import numpy as np

for _p in ("/opt/trn_rl_repo",):
    if os.path.isdir(_p) and _p not in sys.path:
        sys.path.insert(0, _p)

from concourse import bacc, mybir  # noqa: E402
import concourse.bass as bass  # noqa: E402
import concourse.tile as tile  # noqa: E402
from concourse.bass_utils import run_bass_kernel_spmd  # noqa: E402

N_NODES = 50000
N_EDGES = 600000
D_FEAT = 128
N_CORES = 8
NLOC = N_NODES // N_CORES          # 6250 dst nodes per core
BLK = 128                          # dst slots per PSUM block
NB = (NLOC + BLK - 1) // BLK       # 49 blocks per core
HALF = 32768                       # int16 index limit for dma_gather
CALLG = 8                          # groups (of 128 rows) per dma_gather call (>8 overflows the SWDGE ring ucode on HW)
NQ = 4                             # SWDGE queues (ucode MAX_SWDGE_QUEUES)
PAD_NEG1 = True                    # pad with -1 (ucode trims trailing negatives)
MEMSET_INIT = True                 # zero-init gather ring buffers at start
SWDGE_SCRATCH = 16384              # SWDGE descriptor ring: bytes/partition (desc capacity = /16)

_cache = {}


def _prepare(src, dst):
    """Inspector pass: group/pad edges per (core, block, src-half).

    Returns per-core device arrays plus the (core-invariant) group layout.
    """
    core = dst // NLOC
    slot = dst % NLOC
    blk = slot // BLK
    srel = (slot % BLK).astype(np.float32)
    half = (src >= HALF).astype(np.int64)

    cnt = np.zeros((N_CORES, NB, 2), dtype=np.int64)
    np.add.at(cnt, (core, blk, half), 1)
    # groups per (block, half): shared across cores so the SPMD program is identical
    g = (cnt + 127) // 128
    g = g.max(axis=0)  # [NB, 2]
    # every block needs at least one matmul to initialize its PSUM tile
    zero_blocks = g.sum(axis=1) == 0
    g[zero_blocks, 0] = 1

    aoff = np.concatenate([[0], np.cumsum(g[:, 0])])  # A-list group offsets per block
    boff = np.concatenate([[0], np.cumsum(g[:, 1])])
    GA, GB = int(aoff[-1]), int(boff[-1])
    G = GA + GB

    # sort edges by (core, blk, half) once; then slice segments
    key = ((core * NB + blk) * 2 + half)
    order = np.argsort(key, kind="stable")
    key_sorted = key[order]
    src_sorted = src[order]
    srel_sorted = srel[order]
    seg_starts = np.searchsorted(key_sorted, np.arange(N_CORES * NB * 2))
    seg_ends = np.searchsorted(key_sorted, np.arange(N_CORES * NB * 2), side="right")

    # Dummy (padding) edges get index -1. Gather calls are aligned to
    # (block, half) segments, so all padding is trailing within its call:
    # the SWDGE ucode trims trailing negative indices (whole trailing
    # 128-chunks emit no descriptors at all; the rest lanes emit 4-byte
    # dummy descriptors), so padding costs no HBM reads. The gathered-tile
    # region for trimmed slots keeps stale SBUF data, which is harmless:
    # those edges have slot sentinel -1, giving all-zero one-hot rows, and
    # the buffers are memset once at program start so the very first pass
    # cannot multiply garbage NaNs by zero.
    if PAD_NEG1:
        idx_vals = np.full((N_CORES, G, 128), -1, dtype=np.int16)
    else:
        pad = (np.arange(N_CORES * G * 128, dtype=np.int64) * 9973) % 16384
        idx_vals = pad.astype(np.int16).reshape(N_CORES, G, 128)
    slot_vals = np.full((N_CORES, G, 128), -1.0, dtype=np.float32)
    for c in range(N_CORES):
        for b in range(NB):
            for h in range(2):
                s, e = seg_starts[(c * NB + b) * 2 + h], seg_ends[(c * NB + b) * 2 + h]
                n = e - s
                if n == 0:
                    continue
                ng = int(g[b, h])
                g0 = (aoff[b] if h == 0 else GA + boff[b])
                iv = idx_vals[c, g0:g0 + ng].reshape(-1)
                sv = slot_vals[c, g0:g0 + ng].reshape(-1)
                sseg = src_sorted[s:e]
                iv[:n] = (sseg - HALF * h).astype(np.int16)
                sv[:n] = srel_sorted[s:e]

    # wrapped int16 layout for dma_gather: value (g, q) -> [q%16, 8*g + q//16],
    # replicated across the 8 sixteen-partition stripes
    w = idx_vals.reshape(N_CORES, G, 8, 16).transpose(0, 3, 1, 2).reshape(N_CORES, 16, G * 8)
    idxw = np.tile(w, (1, 8, 1))                       # [C, 128, G*8] int16
    slotw = slot_vals.transpose(0, 2, 1).copy()        # [C, 128, G] f32

    # host-side reciprocal degree, laid out [128, NB] per core:
    # rdeg[c, p, b] = 1 / max(deg(dst = c*NLOC + b*128 + p), 1)
    deg = np.bincount(dst, minlength=N_NODES).astype(np.float64)
    rdeg_full = (1.0 / np.maximum(deg, 1.0)).astype(np.float32)
    rdeg = np.zeros((N_CORES, 128, NB), dtype=np.float32)
    for c in range(N_CORES):
        loc = rdeg_full[c * NLOC:(c + 1) * NLOC]
        pad = np.zeros(NB * BLK, dtype=np.float32)
        pad[:NLOC] = loc
        rdeg[c] = pad.reshape(NB, BLK).T

    # per-core post-trim index counts per gather subcall (issue order:
    # for b, for lst in (0,1) with g[b,lst]>0, chunks of CALLG groups).
    # The SWDGE NX decode books ring space from num_idxs_reg while the Q7
    # ucode pushes ceil(trimmed/128) chunks, so the register MUST equal the
    # post-trim count (the -1 padding is trailing per subcall by layout).
    sub = []
    for b in range(NB):
        for h in range(2):
            ncg_total = int(g[b, h])
            if ncg_total == 0:
                continue
            off = 0
            while off < ncg_total:
                ncg = min(CALLG, ncg_total - off)
                sub.append((b, h, off, ncg))
                off += ncg
    NSUB = len(sub)
    nidx = np.zeros((N_CORES, NSUB), dtype=np.int32)
    for j, (b, h, off, ncg) in enumerate(sub):
        n = cnt[:, b, h] - off * 128
        nidx[:, j] = np.clip(n, 0, ncg * 128)

    layout = dict(g=g, aoff=aoff, boff=boff, GA=GA, GB=GB, NSUB=NSUB)
    return idxw, slotw, rdeg, nidx, layout


def _build_program(layout, reps=1):
    """Build the SPMD program. reps>1 unrolls the whole block loop that many
    times back-to-back (same output each rep) — used by the test harness to
    measure steady-state HW time per rep without dispatch-overhead noise."""
    g, aoff, boff = layout["g"], layout["aoff"], layout["boff"]
    GA, GB = layout["GA"], layout["GB"]
    G = GA + GB
    f32 = mybir.dt.float32
    f16 = mybir.dt.float16

    nc = bacc.Bacc("TRN2", target_bir_lowering=False, debug=False,
                   num_devices=N_CORES, dynamic_dma_scratch_size=SWDGE_SCRATCH,
                   num_swdge_queues=NQ)
    # two separate tensors: dma_gather's ucode mishandles nonzero source-AP
    # offsets on HW, so each int16-addressable half gets its own tensor.
    # fp16 rows (256 B) halve gather bytes and matmul/DVE time; exact enough
    # (1e-3 rel) for the 2e-2 gate.
    embA = nc.dram_tensor("embA", [HALF, D_FEAT], f16, kind="ExternalInput").ap()
    embB = nc.dram_tensor("embB", [N_NODES - HALF, D_FEAT], f16, kind="ExternalInput").ap()
    iota = nc.dram_tensor("iota", [128, BLK], f16, kind="ExternalInput").ap()
    idxw = nc.dram_tensor("idxw", [128, G * 8], mybir.dt.int16, kind="ExternalInput").ap()
    slotw = nc.dram_tensor("slotw", [128, G], f16, kind="ExternalInput").ap()
    rdeg = nc.dram_tensor("rdeg", [128, NB], f32, kind="ExternalInput").ap()
    NSUB = layout["NSUB"]
    nidx = nc.dram_tensor("nidx", [128, NSUB], mybir.dt.int32, kind="ExternalInput").ap()
    out = nc.dram_tensor("out", [NLOC, D_FEAT], f32, kind="ExternalOutput").ap()

    # one gather call per (block, half) segment so all padding is trailing
    # within its call and the ucode's trailing-negative-index trim applies.
    # Segments longer than CALLG groups split into CALLG-sized subcalls
    # (>CALLG groups per call overflows the SWDGE ring ucode on HW); the
    # -1 padding lives in the tail groups, i.e. in the last subcall.
    GBUFS = 8

    with tile.TileContext(nc) as tc:
        with (
            tc.tile_pool(name="const", bufs=1) as cpool,
            tc.tile_pool(name="gath", bufs=GBUFS) as gpool,
            tc.tile_pool(name="hbuf", bufs=6) as hpool,
            tc.tile_pool(name="evict", bufs=3) as epool,
            tc.tile_pool(name="psum", bufs=4, space="PSUM") as ppool,
        ):
            iota_sb = cpool.tile([128, BLK], f16, tag="iota")
            nc.sync.dma_start(out=iota_sb[:], in_=iota[:])
            idx_sb = cpool.tile([128, G * 8], mybir.dt.int16, tag="idx")
            nc.sync.dma_start(out=idx_sb[:], in_=idxw[:])
            slot_sb = cpool.tile([128, G], f16, tag="slot")
            nc.sync.dma_start(out=slot_sb[:], in_=slotw[:])
            rdeg_sb = cpool.tile([128, NB], f32, tag="rdeg")
            nc.sync.dma_start(out=rdeg_sb[:], in_=rdeg[:])
            nidx_sb = cpool.tile([128, NSUB], mybir.dt.int32, tag="nidx")
            nc.sync.dma_start(out=nidx_sb[:], in_=nidx[:])

            srcs = {0: embA, 1: embB}
            issue_counter = [0]
            # rotating register set for the per-subcall trim counts (a fresh
            # value_load per call exhausts the 54 allocatable Pool registers)
            nregs = [nc.gpsimd.alloc_register(f"nidx_reg{i}") for i in range(NQ)]
            # subcall index map, mirroring _prepare's enumeration exactly
            subidx = {}
            for b_ in range(NB):
                for h_ in range(2):
                    ncg_total = int(g[b_, h_])
                    off_ = 0
                    while off_ < ncg_total:
                        subidx[(b_, h_, off_)] = len(subidx)
                        off_ += min(CALLG, ncg_total - off_)
            assert len(subidx) == NSUB

            # zero-init every gather ring buffer once so trimmed (stale)
            # regions are finite on the very first pass
            if MEMSET_INIT:
                for lst in (0, 1):
                    for _ in range(GBUFS):
                        tz = gpool.tile([128, CALLG * 128], f16, tag=f"g{lst}")
                        nc.vector.memset(tz[:], 0.0)

            def issue_calls(lst, b):
                """Issue the gather subcalls for segment (b, lst); returns
                the list of [128, <=CALLG*128] tiles in group order."""
                ncg_total = int(g[b, lst])
                grp0 = int(aoff[b]) if lst == 0 else int(boff[b])
                tiles = []
                off = 0
                while off < ncg_total:
                    ncg = min(CALLG, ncg_total - off)
                    t = gpool.tile([128, CALLG * 128], f16, tag=f"g{lst}")
                    col0 = ((grp0 + off) if lst == 0 else (GA + grp0 + off)) * 8
                    q = issue_counter[0] % NQ
                    j = subidx[(b, lst, off)]
                    issue_counter[0] += 1
                    if PAD_NEG1:
                        nreg = nregs[q]
                        nc.gpsimd.reg_load(nreg, nidx_sb[0:1, j:j + 1])
                    else:
                        nreg = ncg * 128
                    nc.gpsimd.dma_gather(
                        out_ap=t[:, :ncg * 128].rearrange("p (n e) -> p n e", e=128),
                        in_ap=srcs[lst],
                        idxs_ap=idx_sb[:, col0:col0 + ncg * 8],
                        num_idxs=ncg * 128,
                        num_idxs_reg=nreg,
                        elem_size=D_FEAT,
                        queue_num=q,
                    )
                    tiles.append(t)
                    off += ncg
                return tiles

            for rep in range(reps):
              for b in range(NB):
                tiles = {}
                for lst in (0, 1):
                    if int(g[b, lst]) > 0:
                        tiles[lst] = issue_calls(lst, b)
                groups = [(0, k) for k in range(int(g[b, 0]))]
                groups += [(1, k) for k in range(int(g[b, 1]))]
                psum_s = ppool.tile([128, BLK], f32, tag="ps")
                last = len(groups) - 1
                for k, (lst, kk) in enumerate(groups):
                    rhs = tiles[lst][kk // CALLG][:, (kk % CALLG) * 128:
                                                  (kk % CALLG + 1) * 128]
                    gg = (int(aoff[b]) if lst == 0 else int(boff[b])) + kk
                    scol = gg if lst == 0 else GA + gg
                    h = hpool.tile([128, BLK], f16, tag="h")
                    nc.vector.tensor_tensor(
                        out=h[:],
                        in0=slot_sb[:, scol:scol + 1].to_broadcast([128, BLK]),
                        in1=iota_sb[:],
                        op=mybir.AluOpType.is_equal,
                    )
                    nc.tensor.matmul(out=psum_s[:], lhsT=h[:], rhs=rhs,
                                     start=(k == 0), stop=(k == last))
                ot = epool.tile([128, BLK], f32, tag="ot")
                nc.vector.tensor_scalar(
                    out=ot[:], in0=psum_s[:], scalar1=rdeg_sb[:, b:b + 1],
                    scalar2=None, op0=mybir.AluOpType.mult,
                )
                rows = min(BLK, NLOC - b * BLK)
                nc.sync.dma_start(out=out[b * BLK:b * BLK + rows, :],
                                  in_=ot[:rows, :])

    nc.compile()
    return nc


def _prepare_all(author_emb, src, dst, reps=1):
    """Host prep shared by kernel() and the test harness: returns
    (nc, in_maps) with the compiled program and per-core input maps."""
    emb = np.ascontiguousarray(np.asarray(author_emb, dtype=np.float32))
    src = np.asarray(src).astype(np.int64)
    dst = np.asarray(dst).astype(np.int64)
    assert emb.shape == (N_NODES, D_FEAT) and src.shape == (N_EDGES,)

    idxw, slotw, rdeg, nidx, layout = _prepare(src, dst)
    key = (layout["GA"], layout["GB"], layout["g"].tobytes(), reps)
    if key not in _cache:
        _cache[key] = _build_program(layout, reps=reps)
    nc = _cache[key]

    iota_np = np.broadcast_to(np.arange(BLK, dtype=np.float16), (128, BLK)).copy()
    embA = np.ascontiguousarray(emb[:HALF].astype(np.float16))
    embB = np.ascontiguousarray(emb[HALF:].astype(np.float16))
    in_maps = [
        {"embA": embA, "embB": embB, "iota": iota_np, "idxw": idxw[c],
         "slotw": slotw[c].astype(np.float16), "rdeg": rdeg[c],
         "nidx": np.broadcast_to(nidx[c], (128, nidx.shape[1])).copy()}
        for c in range(N_CORES)
    ]
    return nc, in_maps


def kernel(author_emb, src, dst, n_nodes):
    nc, in_maps = _prepare_all(author_emb, src, dst)
    res = run_bass_kernel_spmd(nc, in_maps, list(range(N_CORES)))
    out = np.empty((N_NODES, D_FEAT), dtype=np.float32)
    for c in range(N_CORES):
        out[c * NLOC:(c + 1) * NLOC] = res.results[c]["out"]
    return out


# revision 30
# speedup vs baseline: 1.2363x; 1.2363x over previous
"""Trainium2 Bass kernel for GNN copy_src -> segment-mean (dst-sharded, 8 cores).

Strategy
--------
- Partition dst nodes (and their incoming edges) across 8 NeuronCores:
  core c owns dst rows [c*6250, (c+1)*6250).
- Host-side "inspector" pass (numpy), per core and per src-half (the
  50000-row table is addressed as two int16-indexed halves):
  * sort the core's edges by dst block (128 dst slots per block), then
    chunk the sorted list into dense 128-edge groups. Groups are NOT
    aligned to blocks: a group may span 2-3 consecutive blocks, so there
    is no per-(block,half) ceil-padding and no max-over-cores padding —
    only a tiny tail pad per list (<1%). The per-list group count is the
    max over cores so the SPMD program is shared.
  * per (half, block), the host computes the union over cores of the
    group ranges that touch the block; the program does one matmul per
    (block, group-in-range). Groups outside a core's own block range
    contribute zero rows via the one-hot (slot key mismatch).
  * one-hot keys are dst_slot % 1024 (fp16-exact) compared against a
    static [128, 1024] iota table sliced at (block % 8) * 128. A group
    spans <8 blocks so the mod-1024 encoding cannot alias.
  * per-dst reciprocal degree computed host-side.
- Device kernel (identical SPMD program on all 8 cores):
  * dma_gather calls of up to 1024 src rows (256 B fp16 each) from the
    halved tables in HBM into SBUF. Calls round-robin over 4 SWDGE
    queues: each queue's descriptor generation runs on its own Q7 core
    pair, quadrupling descriptor-generation throughput.
  * per (block, half): ONE wide DVE is_equal builds the one-hot for all
    groups in the block's range; fp16 TensorE matmuls accumulate
    H^T @ G into the block's PSUM tile.
  * per block: multiply by the reciprocal-degree column, DMA the
    [128, 128] f32 result tile to the output shard.
- Host gathers the 8 output shards into the full [50000, 128] output.
"""

import os
import sys

import numpy as np

for _p in ("/opt/trn_rl_repo",):
    if os.path.isdir(_p) and _p not in sys.path:
        sys.path.insert(0, _p)

from concourse import bacc, mybir  # noqa: E402
import concourse.bass as bass  # noqa: E402
import concourse.tile as tile  # noqa: E402
from concourse.bass_utils import run_bass_kernel_spmd  # noqa: E402

N_NODES = 50000
N_EDGES = 600000
D_FEAT = 128
N_CORES = 8
NLOC = N_NODES // N_CORES          # 6250 dst nodes per core
BLK = 128                          # dst slots per PSUM block
NB = (NLOC + BLK - 1) // BLK       # 49 blocks per core
HALF = 32768                       # int16 index limit for dma_gather
CALLG = 8                          # groups per dma_gather call (1024 idx is the ucode cap)
NQ = 4                             # SWDGE queues (ucode MAX_SWDGE_QUEUES)
SWDGE_SCRATCH = 16384              # SWDGE descriptor ring: bytes/partition
IOTAW = 1024                       # one-hot key period (block % 8)

_cache = {}


def _prepare(src, dst):
    """Inspector pass: per (core, half) block-sorted dense edge groups.

    Returns per-core device arrays plus the (core-invariant) layout.
    """
    core = dst // NLOC
    slot = dst % NLOC
    blk = slot // BLK
    smod = (slot % IOTAW).astype(np.float16)
    half = (src >= HALF).astype(np.int64)

    # counts per (core, half, block)
    cnt = np.zeros((N_CORES, 2, NB), dtype=np.int64)
    np.add.at(cnt, (core, half, blk), 1)
    n_ch = cnt.sum(axis=2)                      # [C, 2] edges per (core, half)
    G_h = [int(np.ceil(n_ch[:, h].max() / 128)) for h in (0, 1)]
    GA, GB = G_h[0], G_h[1]
    G = GA + GB

    # per (half, block): union group range over cores
    cum = np.concatenate([np.zeros((N_CORES, 2, 1), dtype=np.int64),
                          np.cumsum(cnt, axis=2)], axis=2)  # [C,2,NB+1]
    ranges = np.zeros((2, NB, 2), dtype=np.int64)
    for h in (0, 1):
        for b in range(NB):
            has = cnt[:, h, b] > 0
            if has.any():
                lo = (cum[has, h, b] // 128).min()
                hi = ((cum[has, h, b + 1] + 127) // 128).max()
            else:
                lo = int((cum[:, h, b] // 128).min())
                lo = min(lo, G_h[h] - 1)
                hi = lo + 1 if h == 0 else lo  # empty B-range allowed
            ranges[h, b] = (lo, hi)

    # sort edges by (core, half, blk) once; slice per-core-half segments
    key = (core * 2 + half) * NB + blk
    order = np.argsort(key, kind="stable")
    src_sorted = src[order]
    smod_sorted = smod[order]
    blk_sorted = blk[order]
    ch_key = (core * 2 + half)[order]
    seg_starts = np.searchsorted(ch_key, np.arange(N_CORES * 2))
    seg_ends = np.searchsorted(ch_key, np.arange(N_CORES * 2), side="right")

    idx_vals = np.zeros((N_CORES, G, 128), dtype=np.int16)
    # spread padding indices across the table (any valid row; slot key -1
    # gives an all-zero one-hot row so the data is discarded)
    pad = ((np.arange(N_CORES * G * 128, dtype=np.int64) * 9973) % 16384)
    idx_vals[:] = pad.astype(np.int16).reshape(N_CORES, G, 128)
    slot_vals = np.full((N_CORES, G, 128), -1.0, dtype=np.float16)

    for c in range(N_CORES):
        for h in (0, 1):
            s, e = seg_starts[c * 2 + h], seg_ends[c * 2 + h]
            n = e - s
            if n == 0:
                continue
            g0 = 0 if h == 0 else GA
            iv = idx_vals[c, g0:g0 + G_h[h]].reshape(-1)
            sv = slot_vals[c, g0:g0 + G_h[h]].reshape(-1)
            iv[:n] = (src_sorted[s:e] - HALF * h).astype(np.int16)
            sv[:n] = smod_sorted[s:e]
            # mod-1024 alias safety: a group must span < 8 blocks
            bseg = blk_sorted[s:e]
            ng = (n + 127) // 128
            bpad = np.concatenate([bseg, np.full(ng * 128 - n, bseg[-1])])
            bg = bpad.reshape(ng, 128)
            assert int((bg.max(axis=1) - bg.min(axis=1)).max()) < 8

    # wrapped int16 layout for dma_gather: value (g, q) -> [q%16, 8*g + q//16]
    w = idx_vals.reshape(N_CORES, G, 8, 16).transpose(0, 3, 1, 2).reshape(N_CORES, 16, G * 8)
    idxw = np.tile(w, (1, 8, 1))                       # [C, 128, G*8] int16
    slotw = slot_vals.transpose(0, 2, 1).copy()        # [C, 128, G] f16

    # host-side reciprocal degree, laid out [128, NB] per core
    deg = np.bincount(dst, minlength=N_NODES).astype(np.float64)
    rdeg_full = (1.0 / np.maximum(deg, 1.0)).astype(np.float32)
    rdeg = np.zeros((N_CORES, 128, NB), dtype=np.float32)
    for c in range(N_CORES):
        padr = np.zeros(NB * BLK, dtype=np.float32)
        padr[:NLOC] = rdeg_full[c * NLOC:(c + 1) * NLOC]
        rdeg[c] = padr.reshape(NB, BLK).T

    layout = dict(GA=GA, GB=GB, ranges=ranges)
    return idxw, slotw, rdeg, layout


def _build_program(layout, reps=1):
    """Build the SPMD program. reps>1 unrolls the whole body that many
    times back-to-back (same output each rep) — used by the test harness to
    measure steady-state HW time per rep without dispatch-overhead noise."""
    GA, GB = layout["GA"], layout["GB"]
    ranges = layout["ranges"]
    G = GA + GB
    G_h = {0: GA, 1: GB}
    f32 = mybir.dt.float32
    f16 = mybir.dt.float16

    nc = bacc.Bacc("TRN2", target_bir_lowering=False, debug=False,
                   num_devices=N_CORES, dynamic_dma_scratch_size=SWDGE_SCRATCH,
                   num_swdge_queues=NQ)
    embA = nc.dram_tensor("embA", [HALF, D_FEAT], f16, kind="ExternalInput").ap()
    embB = nc.dram_tensor("embB", [N_NODES - HALF, D_FEAT], f16, kind="ExternalInput").ap()
    iota = nc.dram_tensor("iota", [128, IOTAW], f16, kind="ExternalInput").ap()
    idxw = nc.dram_tensor("idxw", [128, G * 8], mybir.dt.int16, kind="ExternalInput").ap()
    slotw = nc.dram_tensor("slotw", [128, G], f16, kind="ExternalInput").ap()
    rdeg = nc.dram_tensor("rdeg", [128, NB], f32, kind="ExternalInput").ap()
    out = nc.dram_tensor("out", [NLOC, D_FEAT], f32, kind="ExternalOutput").ap()

    ncalls = {h: (G_h[h] + CALLG - 1) // CALLG for h in (0, 1)}
    GBUFS = 8

    with tile.TileContext(nc) as tc:
        with (
            tc.tile_pool(name="const", bufs=1) as cpool,
            tc.tile_pool(name="gath", bufs=GBUFS) as gpool,
            tc.tile_pool(name="hbuf", bufs=6) as hpool,
            tc.tile_pool(name="evict", bufs=3) as epool,
            tc.tile_pool(name="psum", bufs=4, space="PSUM") as ppool,
        ):
            iota_sb = cpool.tile([128, IOTAW], f16, tag="iota")
            nc.sync.dma_start(out=iota_sb[:], in_=iota[:])
            idx_sb = cpool.tile([128, G * 8], mybir.dt.int16, tag="idx")
            nc.sync.dma_start(out=idx_sb[:], in_=idxw[:])
            slot_sb = cpool.tile([128, G], f16, tag="slot")
            nc.sync.dma_start(out=slot_sb[:], in_=slotw[:])
            rdeg_sb = cpool.tile([128, NB], f32, tag="rdeg")
            nc.sync.dma_start(out=rdeg_sb[:], in_=rdeg[:])

            srcs = {0: embA, 1: embB}
            issue_counter = [0]
            call_tiles = {}
            next_call = {0: 0, 1: 0}

            def ensure_call(h, ci):
                """Issue gather calls of list h up to and including ci."""
                while next_call[h] <= ci:
                    c0 = next_call[h]
                    ncg = min(CALLG, G_h[h] - c0 * CALLG)
                    t = gpool.tile([128, CALLG * 128], f16, tag=f"g{h}")
                    col0 = (c0 * CALLG + (0 if h == 0 else GA)) * 8
                    q = issue_counter[0] % NQ
                    issue_counter[0] += 1
                    nc.gpsimd.dma_gather(
                        out_ap=t[:, :ncg * 128].rearrange("p (n e) -> p n e", e=128),
                        in_ap=srcs[h],
                        idxs_ap=idx_sb[:, col0:col0 + ncg * 8],
                        num_idxs=ncg * 128,
                        num_idxs_reg=ncg * 128,
                        elem_size=D_FEAT,
                        queue_num=q,
                    )
                    call_tiles[(h, c0)] = t
                    next_call[h] += 1

            for rep in range(reps):
                call_tiles.clear()
                next_call[0] = next_call[1] = 0
                for b in range(NB):
                    # (half, group-range) pairs for this block
                    work = []
                    for h in (0, 1):
                        rs, re = int(ranges[h, b, 0]), int(ranges[h, b, 1])
                        if re > rs:
                            work.append((h, rs, re))
                    total = sum(re - rs for _, rs, re in work)
                    psum_s = ppool.tile([128, BLK], f32, tag="ps")
                    islice = iota_sb[:, (b % 8) * BLK:(b % 8 + 1) * BLK] \
                        .rearrange("p (o s) -> p o s", o=1)
                    k = 0
                    for h, rs, re in work:
                        gg = rs
                        while gg < re:
                            ci = gg // CALLG
                            ce = min(re, (ci + 1) * CALLG)
                            ng = ce - gg
                            ensure_call(h, ci)
                            t = call_tiles[(h, ci)]
                            scol = gg + (0 if h == 0 else GA)
                            hw = hpool.tile([128, CALLG, BLK], f16, tag="hw")
                            nc.vector.tensor_tensor(
                                out=hw[:, :ng, :],
                                in0=slot_sb[:, scol:scol + ng]
                                    .to_broadcast([128, ng, BLK]),
                                in1=islice.to_broadcast([128, ng, BLK]),
                                op=mybir.AluOpType.is_equal,
                            )
                            for kk in range(ng):
                                loc = (gg + kk) % CALLG
                                rhs = t[:, loc * 128:(loc + 1) * 128]
                                nc.tensor.matmul(
                                    out=psum_s[:], lhsT=hw[:, kk, :], rhs=rhs,
                                    start=(k == 0), stop=(k == total - 1))
                                k += 1
                            gg = ce
                    ot = epool.tile([128, BLK], f32, tag="ot")
                    nc.vector.tensor_scalar(
                        out=ot[:], in0=psum_s[:], scalar1=rdeg_sb[:, b:b + 1],
                        scalar2=None, op0=mybir.AluOpType.mult,
                    )
                    rows = min(BLK, NLOC - b * BLK)
                    nc.sync.dma_start(out=out[b * BLK:b * BLK + rows, :],
                                      in_=ot[:rows, :])

    nc.compile()
    return nc


def _prepare_all(author_emb, src, dst, reps=1):
    """Host prep shared by kernel() and the test harness: returns
    (nc, in_maps) with the compiled program and per-core input maps."""
    emb = np.ascontiguousarray(np.asarray(author_emb, dtype=np.float32))
    src = np.asarray(src).astype(np.int64)
    dst = np.asarray(dst).astype(np.int64)
    assert emb.shape == (N_NODES, D_FEAT) and src.shape == (N_EDGES,)

    idxw, slotw, rdeg, layout = _prepare(src, dst)
    key = (layout["GA"], layout["GB"], layout["ranges"].tobytes(), reps)
    if key not in _cache:
        _cache[key] = _build_program(layout, reps=reps)
    nc = _cache[key]

    iota_np = np.broadcast_to(np.arange(IOTAW, dtype=np.float16), (128, IOTAW)).copy()
    embA = np.ascontiguousarray(emb[:HALF].astype(np.float16))
    embB = np.ascontiguousarray(emb[HALF:].astype(np.float16))
    in_maps = [
        {"embA": embA, "embB": embB, "iota": iota_np, "idxw": idxw[c],
         "slotw": slotw[c], "rdeg": rdeg[c]}
        for c in range(N_CORES)
    ]
    return nc, in_maps


def kernel(author_emb, src, dst, n_nodes):
    nc, in_maps = _prepare_all(author_emb, src, dst)
    res = run_bass_kernel_spmd(nc, in_maps, list(range(N_CORES)))
    out = np.empty((N_NODES, D_FEAT), dtype=np.float32)
    for c in range(N_CORES):
        out[c * NLOC:(c + 1) * NLOC] = res.results[c]["out"]
    return out
